# revision 5
# baseline (speedup 1.0000x reference)
"""Trainium2 Bass kernel for nn_MultiHeadAttention_47382079209593.

Full-input contract: kernel(**inputs) takes the complete unsharded tensors and
returns the full (out, decomposed) pair, distributing work across 8 NeuronCores
internally.

Sharding:
  - Attention (qkv proj, softmax, out proj): data-parallel over batch, 8
    batches per core.
  - decomposed = (out[:, -1, :] @ W_ctx): column-parallel over W_ctx's
    512*512 output dim -> core i owns block positions w in [64i, 64i+64) for
    ALL 64 batches.  The 64x512 last-token activations are shared via an
    on-device AllGather (16 KB per core).
  - decomposed2 = (prev + dec) @ W_proj2: row-parallel over the (b, w) dim,
    no communication needed.

All heavy matmuls run in float32r (full-rate fp32 PE mode, ~1.6e-4 rel err).
"""

import sys

if '/opt/trn_rl_repo' not in sys.path:
    sys.path.insert(0, '/opt/trn_rl_repo')

import numpy as np

import concourse.bass as bass
import concourse.tile as tile
from concourse import bacc, mybir
from concourse.bass_utils import run_bass_kernel_spmd

F32 = mybir.dt.float32
F32R = mybir.dt.float32r
EXP = mybir.ActivationFunctionType.Exp

B, W, C = 64, 512, 512
H = 8
DH = C // H          # 64
BLOCK = 512
N_CORES = 8
BPC = B // N_CORES   # 8 batches per core
WPC = BLOCK // N_CORES  # 64 block positions per core
MASK_NEG = -30000.0


def r(ap):
    return ap.bitcast(F32R)


def build_kernel():
    nc = bacc.Bacc("TRN2", num_devices=N_CORES)

    x_ext = nc.dram_tensor("x", [BPC, W, C], F32, kind="ExternalInput")
    prev_ext = nc.dram_tensor("prev", [B, WPC, C], F32, kind="ExternalInput")
    wattn_ext = nc.dram_tensor("w_attn", [C, 3 * C], F32, kind="ExternalInput")
    wctx_ext = nc.dram_tensor("w_ctx", [C, WPC * C], F32, kind="ExternalInput")
    wproj_ext = nc.dram_tensor("w_proj", [C, C], F32, kind="ExternalInput")
    wproj2_ext = nc.dram_tensor("w_proj2", [C, C], F32, kind="ExternalInput")
    ident_ext = nc.dram_tensor("ident", [128, 128], F32, kind="ExternalInput")
    maskt_ext = nc.dram_tensor("maskt", [128, 128], F32, kind="ExternalInput")
    ones_ext = nc.dram_tensor("ones", [128, 1], F32, kind="ExternalInput")

    out_ext = nc.dram_tensor("out", [BPC, W, C], F32, kind="ExternalOutput")
    dec_ext = nc.dram_tensor("dec", [B, WPC, C], F32, kind="ExternalOutput")

    cc_in = nc.dram_tensor("cc_in", [BPC, C], F32)
    cc_out = nc.dram_tensor("cc_out", [B, C], F32, addr_space="Shared")

    from contextlib import ExitStack

    with tile.TileContext(nc) as tc, ExitStack() as ctx:
        if True:
            pool = lambda name, bufs, **kw: ctx.enter_context(
                tc.tile_pool(name=name, bufs=bufs, **kw))
            consts = pool("consts", 1)
            weights = pool("weights", 1)
            persist = pool("persist", 1)
            # PSUM pools: 8 banks total
            ps_mm = pool("ps_mm", 2, space="PSUM")
            ps_sc = pool("ps_sc", 2, space="PSUM")
            ps_ot = pool("ps_ot", 2, space="PSUM")
            ps_sum = pool("ps_sum", 1, space="PSUM")
            ps_xp = pool("ps_xp", 1, space="PSUM")
            p_x = pool("p_x", 1)
            p_xt = pool("p_xt", 1)
            p_qkt = pool("p_qkt", 1)
            p_v = pool("p_v", 2)
            p_exp = pool("p_exp", 2)
            p_out = pool("p_out", 1)
            p_small = pool("p_small", 2)
            p_cp = pool("p_cp", 3)
            p_wc = pool("p_wc", 2)
            p_dec = pool("p_dec", 2)

            # ---- constants & weights ----
            ident = consts.tile([128, 128], F32)
            nc.sync.dma_start(out=ident[:], in_=ident_ext[:])
            maskt = consts.tile([128, 128], F32)
            nc.sync.dma_start(out=maskt[:], in_=maskt_ext[:])
            ones = consts.tile([128, 1], F32)
            nc.sync.dma_start(out=r(ones[:]), in_=r(ones_ext[:]))

            wattn = weights.tile([128, 4, 3 * C], F32)
            for kc in range(4):
                nc.sync.dma_start(
                    out=r(wattn[:, kc, :]),
                    in_=r(wattn_ext[kc * 128:(kc + 1) * 128, :]))
            wproj = weights.tile([64, H, C], F32)
            for h in range(H):
                nc.sync.dma_start(
                    out=r(wproj[:, h, :]),
                    in_=r(wproj_ext[h * 64:(h + 1) * 64, :]))
            wproj2 = weights.tile([128, 4, C], F32)
            for kc in range(4):
                nc.sync.dma_start(
                    out=r(wproj2[:, kc, :]),
                    in_=r(wproj2_ext[kc * 128:(kc + 1) * 128, :]))

            lastT = persist.tile([64, H], F32)  # staging of out_last^T per batch

            # ================= attention phase (per local batch) ============
            for b in range(BPC):
                # load x_b [4 tok-chunks, 128, 512]
                x_sb = p_x.tile([128, 4, C], F32)
                for t in range(4):
                    nc.sync.dma_start(
                        out=x_sb[:, t, :],
                        in_=x_ext[b, t * 128:(t + 1) * 128, :])
                # transpose -> xT [128, cc, tok]
                xt_sb = p_xt.tile([128, 4, W], F32)
                for t in range(4):
                    for cc in range(4):
                        xp = ps_xp.tile([128, 128], F32, tag="xp")
                        nc.tensor.transpose(
                            xp[:], x_sb[:, t, cc * 128:(cc + 1) * 128], ident[:])
                        nc.vector.tensor_copy(
                            r(xt_sb[:, cc, t * 128:(t + 1) * 128]), xp[:])

                # qkT [128, mc(8), tok]  (rows of (x@Wqk)^T)
                qkt = p_qkt.tile([128, 8, W], F32)
                for mc in range(8):
                    ps = ps_mm.tile([128, W], F32, tag="mm")
                    for kc in range(4):
                        nc.tensor.matmul(
                            ps[:],
                            r(wattn[:, kc, mc * 128:(mc + 1) * 128]),
                            r(xt_sb[:, kc, :]),
                            start=(kc == 0), stop=(kc == 3))
                    nc.vector.tensor_copy(r(qkt[:, mc, :]), ps[:])

                # v natural [128(tok), t(4), 512]
                v_sb = p_v.tile([128, 4, C], F32)
                for t in range(4):
                    ps = ps_mm.tile([128, C], F32, tag="mm")
                    for kc in range(4):
                        nc.tensor.matmul(
                            ps[:],
                            r(xt_sb[:, kc, t * 128:(t + 1) * 128]),
                            r(wattn[:, kc, 2 * C:3 * C]),
                            start=(kc == 0), stop=(kc == 3))
                    nc.vector.tensor_copy(r(v_sb[:, t, :]), ps[:])

                outt = p_out.tile([64, H, W], F32)  # normalized outT per head

                for h in range(H):
                    base = (h % 2) * 64
                    mq = h // 2
                    mk = 4 + h // 2
                    qt = qkt[base:base + 64, mq, :]
                    kt = qkt[base:base + 64, mk, :]

                    et = p_exp.tile([128, 4, W], F32)
                    sums = ps_sum.tile([1, W], F32, tag="sum")
                    ot = ps_ot.tile([64, W], F32, tag="ot")
                    for ki in range(4):
                        n = W - ki * 128
                        q0 = ki * 128
                        sc = ps_sc.tile([128, W], F32, tag="sc")
                        nc.tensor.matmul(
                            sc[:, :n],
                            r(kt[:, ki * 128:(ki + 1) * 128]),
                            r(qt[:, q0:]),
                            start=True, stop=True)
                        # causal mask on the diagonal block (first 128 cols)
                        nc.vector.tensor_add(sc[:, :128], sc[:, :128], maskt[:])
                        nc.scalar.activation(
                            r(et[:, ki, :n]), sc[:, :n], EXP, scale=0.125)
                        nc.tensor.matmul(
                            sums[0:1, q0:], r(ones[:]), r(et[:, ki, :n]),
                            start=(ki == 0), stop=(ki == 3))
                        nc.tensor.matmul(
                            ot[0:64, q0:],
                            r(v_sb[:, ki, h * 64:(h + 1) * 64]),
                            r(et[:, ki, :n]),
                            start=(ki == 0), stop=(ki == 3))

                    recip = p_small.tile([1, W], F32)
                    nc.vector.reciprocal(recip[:], sums[0:1, :])
                    bcast = p_small.tile([64, W], F32)
                    nc.gpsimd.partition_broadcast(bcast[:], recip[:])
                    nc.vector.tensor_mul(r(outt[:, h, :]), ot[0:64, :], bcast[:])

                # stage out_last^T columns: lastT[d, h] = outT[d, h, 511]
                nc.vector.tensor_copy(lastT[:, :], outt[:, :, W - 1])

                # out proj: out[tok, :] = sum_h outT[:, h, tok].T @ Wproj[h]
                for t in range(4):
                    ps = ps_mm.tile([128, C], F32, tag="mm")
                    for h in range(H):
                        nc.tensor.matmul(
                            ps[:],
                            r(outt[:, h, t * 128:(t + 1) * 128]),
                            r(wproj[:, h, :]),
                            start=(h == 0), stop=(h == 7))
                    pr = p_cp.tile([128, C], F32)
                    nc.vector.tensor_copy(pr[:], ps[:])
                    nc.sync.dma_start(
                        out=out_ext[b, t * 128:(t + 1) * 128, :], in_=pr[:])

                # out_last natural row for this batch -> cc_in[b, h*64+d]
                cc_ap = cc_in[:]
                nc.sync.dma_start(
                    out=bass.AP(tensor=cc_ap.tensor, offset=b * C,
                                ap=[[1, 64], [64, H]]),
                    in_=lastT[:, :])

            # ================= collective =================
            nc.gpsimd.collective_compute(
                "AllGather",
                mybir.AluOpType.bypass,
                ins=[cc_in[:]],
                outs=[cc_out[:]],
                replica_groups=[list(range(N_CORES))],
            )

            # ================= decomposed phase =================
            ol = p_dec.tile([64, C], F32)  # out_last [64 batches, 512]
            nc.sync.dma_start(out=ol[:], in_=cc_out[:])
            lastT_all = persist.tile([128, 4, 64], F32)
            for t in range(4):
                xp = ps_xp.tile([128, 64], F32, tag="xp")
                nc.tensor.transpose(
                    xp[:], ol[:, t * 128:(t + 1) * 128], ident[0:64, 0:64])
                nc.vector.tensor_copy(r(lastT_all[:, t, :]), xp[:])

            for w in range(WPC):
                wc = p_wc.tile([128, 4, C], F32)
                for kc in range(4):
                    nc.sync.dma_start(
                        out=r(wc[:, kc, :]),
                        in_=r(wctx_ext[kc * 128:(kc + 1) * 128,
                                       w * C:(w + 1) * C]))
                dps = ps_mm.tile([64, C], F32, tag="mm")
                for kc in range(4):
                    nc.tensor.matmul(
                        dps[0:64, :], r(lastT_all[:, kc, :]), r(wc[:, kc, :]),
                        start=(kc == 0), stop=(kc == 3))
                pv = p_dec.tile([64, C], F32)
                nc.sync.dma_start(out=pv[:], in_=prev_ext[:, w, :])
                s_sb = p_dec.tile([64, C], F32)
                nc.vector.tensor_add(s_sb[:], dps[0:64, :], pv[:])
                st = p_dec.tile([128, 4, 64], F32)
                for t in range(4):
                    xp = ps_xp.tile([128, 64], F32, tag="xp")
                    nc.tensor.transpose(
                        xp[:], s_sb[:, t * 128:(t + 1) * 128],
                        ident[0:64, 0:64])
                    nc.vector.tensor_copy(r(st[:, t, :]), xp[:])
                d2 = ps_mm.tile([64, C], F32, tag="mm")
                for t in range(4):
                    nc.tensor.matmul(
                        d2[0:64, :], r(st[:, t, :]), r(wproj2[:, t, :]),
                        start=(t == 0), stop=(t == 3))
                d2s = p_dec.tile([64, C], F32)
                nc.vector.tensor_copy(d2s[:], d2[0:64, :])
                nc.sync.dma_start(out=dec_ext[:, w, :], in_=d2s[:])

    nc.finalize()
    return nc


_NC_CACHE = None


def _get_nc():
    global _NC_CACHE
    if _NC_CACHE is None:
        _NC_CACHE = build_kernel()
    return _NC_CACHE


def make_in_maps(x, prev_decomposed, W_attn, W_ctx, W_proj, W_proj2):
    ident = np.eye(128, dtype=np.float32)
    # scoresT layout [k, q]: mask out k > q within the diagonal block
    kk, qq = np.meshgrid(np.arange(128), np.arange(128), indexing="ij")
    maskt = np.where(kk > qq, np.float32(MASK_NEG), np.float32(0.0))
    ones = np.ones((128, 1), dtype=np.float32)

    in_maps = []
    for i in range(N_CORES):
        in_maps.append({
            "x": np.ascontiguousarray(x[i * BPC:(i + 1) * BPC]),
            "prev": np.ascontiguousarray(
                prev_decomposed[:, i * WPC:(i + 1) * WPC, :]),
            "w_attn": np.ascontiguousarray(W_attn),
            "w_ctx": np.ascontiguousarray(
                W_ctx[:, i * WPC * C:(i + 1) * WPC * C]),
            "w_proj": np.ascontiguousarray(W_proj),
            "w_proj2": np.ascontiguousarray(W_proj2),
            "ident": ident,
            "maskt": maskt,
            "ones": ones,
        })
    return in_maps


def run(x, prev_decomposed, W_attn, W_ctx, W_proj, W_proj2, **spmd_kwargs):
    nc = _get_nc()
    in_maps = make_in_maps(x, prev_decomposed, W_attn, W_ctx, W_proj, W_proj2)
    res = run_bass_kernel_spmd(nc, in_maps, list(range(N_CORES)), **spmd_kwargs)
    results = res.results
    out = np.concatenate(
        [np.asarray(results[i]["out"]) for i in range(N_CORES)], axis=0)
    dec = np.concatenate(
        [np.asarray(results[i]["dec"]) for i in range(N_CORES)], axis=1)
    return (out, dec), res


def kernel(x, prev_decomposed, W_attn, W_ctx, W_proj, W_proj2):
    (out, dec), _ = run(
        np.asarray(x, dtype=np.float32),
        np.asarray(prev_decomposed, dtype=np.float32),
        np.asarray(W_attn, dtype=np.float32),
        np.asarray(W_ctx, dtype=np.float32),
        np.asarray(W_proj, dtype=np.float32),
        np.asarray(W_proj2, dtype=np.float32))
    return (out, dec)


# revision 18
# speedup vs baseline: 1.2424x; 1.2424x over previous
"""Trainium2 Bass kernel for nn_MultiHeadAttention_47382079209593.

Full-input contract: kernel(**inputs) takes the complete unsharded tensors and
returns the full (out, decomposed) pair, distributing work across 8 NeuronCores
internally.

Sharding:
  - Attention (qkv proj, softmax, out proj): data-parallel over batch, 8
    batches per core.
  - decomposed = (out[:, -1, :] @ W_ctx): column-parallel over W_ctx's
    512*512 output dim -> core i owns block positions w in [64i, 64i+64) for
    ALL 64 batches.  The 64x512 last-token activations are shared via an
    on-device AllGather (16 KB per core).
  - decomposed2 = (prev + dec) @ W_proj2: row-parallel over the (b, w) dim,
    no communication needed.

All heavy matmuls run in float32r (full-rate fp32 PE mode, ~1.6e-4 rel err).
"""

import sys

if '/opt/trn_rl_repo' not in sys.path:
    sys.path.insert(0, '/opt/trn_rl_repo')

import numpy as np

import concourse.bass as bass
import concourse.tile as tile
from concourse import bacc, mybir
from concourse.bass_utils import run_bass_kernel_spmd

F32 = mybir.dt.float32
BF16 = mybir.dt.bfloat16
F32R = mybir.dt.float32r
EXP = mybir.ActivationFunctionType.Exp

B, W, C = 64, 512, 512
H = 8
DH = C // H          # 64
BLOCK = 512
N_CORES = 8
BPC = B // N_CORES   # 8 batches per core
WPC = BLOCK // N_CORES  # 64 block positions per core



def r(ap):
    return ap.bitcast(F32R)


def build_kernel():
    nc = bacc.Bacc("TRN2", num_devices=N_CORES)

    x_ext = nc.dram_tensor("x", [BPC, W, C], F32, kind="ExternalInput")
    prev_ext = nc.dram_tensor("prev", [B, WPC, C], F32, kind="ExternalInput")
    wattn_ext = nc.dram_tensor("w_attn", [C, 3 * C], F32, kind="ExternalInput")
    wctx_ext = nc.dram_tensor("w_ctx", [C, WPC * C], BF16, kind="ExternalInput")
    wproj_ext = nc.dram_tensor("w_proj", [C, C], F32, kind="ExternalInput")
    wproj2_ext = nc.dram_tensor("w_proj2", [C, C], F32, kind="ExternalInput")
    ident_ext = nc.dram_tensor("ident", [128, 128], F32, kind="ExternalInput")
    maskt_ext = nc.dram_tensor("maskt", [128, 128], F32, kind="ExternalInput")  # 0/1 keep-mask
    ones_ext = nc.dram_tensor("ones", [128, 1], F32, kind="ExternalInput")

    out_ext = nc.dram_tensor("out", [BPC, W, C], F32, kind="ExternalOutput")
    dec_ext = nc.dram_tensor("dec", [B, WPC, C], F32, kind="ExternalOutput")

    cc_in = nc.dram_tensor("cc_in", [BPC, C], F32)
    cc_out = nc.dram_tensor("cc_out", [B, C], F32, addr_space="Shared")

    from contextlib import ExitStack

    with tile.TileContext(nc) as tc, ExitStack() as ctx:
        if True:
            pool = lambda name, bufs, **kw: ctx.enter_context(
                tc.tile_pool(name=name, bufs=bufs, **kw))
            consts = pool("consts", 1)
            weights = pool("weights", 1)
            persist = pool("persist", 1)
            # PSUM pools: 8 banks total
            ps_mm = pool("ps_mm", 2, space="PSUM")
            ps_sc = pool("ps_sc", 3, space="PSUM")
            ps_ot = pool("ps_ot", 2, space="PSUM")
            ps_xp = pool("ps_xp", 1, space="PSUM")
            p_x = pool("p_x", 1)
            p_xt = pool("p_xt", 1)
            p_qkt = pool("p_qkt", 1)
            p_v = pool("p_v", 2)
            p_exp = pool("p_exp", 2)
            p_out = pool("p_out", 1)
            p_small = pool("p_small", 2)
            p_cp = pool("p_cp", 3)
            p_wc = pool("p_wc", 4)
            p_dec = pool("p_dec", 2)

            # ---- constants & weights ----
            ident = consts.tile([128, 128], F32)
            nc.sync.dma_start(out=ident[:], in_=ident_ext[:])
            maskt = consts.tile([128, 128], F32)
            nc.sync.dma_start(out=maskt[:], in_=maskt_ext[:])
            ones = consts.tile([128, 1], F32)
            nc.sync.dma_start(out=r(ones[:]), in_=r(ones_ext[:]))

            wattn = weights.tile([128, 4, 3 * C], F32)
            nc.sync.dma_start(
                out=r(wattn[:]),
                in_=r(wattn_ext[:].rearrange("(k p) c -> p k c", p=128)))
            wproj = weights.tile([64, H, C], F32)
            nc.sync.dma_start(
                out=r(wproj[:]),
                in_=r(wproj_ext[:].rearrange("(h p) c -> p h c", p=64)))
            wproj2 = weights.tile([128, 4, C], F32)
            nc.sync.dma_start(
                out=r(wproj2[:]),
                in_=r(wproj2_ext[:].rearrange("(k p) c -> p k c", p=128)))

            lastT = persist.tile([64, H], F32)  # staging of out_last^T per batch

            # ================= attention phase (per local batch) ============
            for b in range(BPC):
                # load x_b [4 tok-chunks, 128, 512]
                x_sb = p_x.tile([128, 4, C], F32)
                nc.sync.dma_start(
                    out=x_sb[:],
                    in_=x_ext[b].rearrange("(t p) c -> p t c", p=128))
                # transpose -> xT [128, cc, tok]
                xt_sb = p_xt.tile([128, 4, W], F32)
                for cc in range(4):
                    xp = ps_xp.tile([128, W], F32, tag="xp")
                    for t in range(4):
                        nc.tensor.transpose(
                            xp[:, t * 128:(t + 1) * 128],
                            x_sb[:, t, cc * 128:(cc + 1) * 128], ident[:])
                    nc.vector.tensor_copy(r(xt_sb[:, cc, :]), xp[:])

                # qkT [128, mc(8), tok]  (rows of (x@Wqk)^T)
                qkt = p_qkt.tile([128, 8, W], F32)
                for mc in (0, 4, 1, 5, 2, 6, 3, 7):
                    ps = ps_mm.tile([128, W], F32, tag="mm")
                    for kc in range(4):
                        nc.tensor.matmul(
                            ps[:],
                            r(wattn[:, kc, mc * 128:(mc + 1) * 128]),
                            r(xt_sb[:, kc, :]),
                            start=(kc == 0), stop=(kc == 3))
                    nc.vector.tensor_copy(r(qkt[:, mc, :]), ps[:])

                # v natural + ones col per head: [128(tok), t(4), h(8), 65]
                v_sb = p_v.tile([128, 4, H, 65], F32)
                for t in range(4):
                    ps = ps_mm.tile([128, C], F32, tag="mm")
                    for kc in range(4):
                        nc.tensor.matmul(
                            ps[:],
                            r(xt_sb[:, kc, t * 128:(t + 1) * 128]),
                            r(wattn[:, kc, 2 * C:3 * C]),
                            start=(kc == 0), stop=(kc == 3))
                    nc.vector.tensor_copy(
                        r(v_sb[:, t, :, 0:64]),
                        ps[:].rearrange("p (h d) -> p h d", h=H))
                    nc.vector.memset(v_sb[:, t, :, 64], 1.0)

                outt = p_out.tile([64, H, W], F32)  # normalized outT per head

                def make_head(h, et, ot):
                    base = (h % 2) * 64
                    qt = qkt[base:base + 64, h // 2, :]
                    kt = qkt[base:base + 64, 4 + h // 2, :]

                    def scores_strip(ki):
                        n = W - ki * 128
                        sc = ps_sc.tile([128, W], F32, tag="sc")
                        nc.tensor.matmul(
                            sc[:, :n],
                            r(kt[:, ki * 128:(ki + 1) * 128]),
                            r(qt[:, ki * 128:]),
                            start=True, stop=True)
                        nc.scalar.activation(
                            r(et[:, ki, :n]), sc[:, :n], EXP, scale=0.125)
                        # causal 0/1 mask on the diagonal block
                        nc.vector.tensor_mul(
                            r(et[:, ki, :128]), r(et[:, ki, :128]), maskt[:])

                    def attnv_strip(ki):
                        n = W - ki * 128
                        nc.tensor.matmul(
                            ot[0:65, ki * 128:],
                            r(v_sb[:, ki, h, :]),
                            r(et[:, ki, :n]),
                            start=(ki == 0), stop=(ki == 3))

                    def finish():
                        recip = p_small.tile([1, W], F32)
                        nc.vector.reciprocal(recip[:], ot[64:65, :])
                        bcast = p_small.tile([64, W], F32)
                        nc.gpsimd.partition_broadcast(bcast[:], recip[:])
                        nc.vector.tensor_mul(
                            r(outt[:, h, :]), ot[0:64, :], bcast[:])

                    return scores_strip, attnv_strip, finish

                # heads in pairs: even head uses partitions 0-63 (PE rows
                # 0-63), odd head rows 64-127 -> score matmuls of the pair
                # land on disjoint PE row groups and can overlap.
                for hp in range(4):
                    h0, h1 = 2 * hp, 2 * hp + 1
                    et0 = p_exp.tile([128, 4, W], F32, tag="et")
                    ot0 = ps_ot.tile([65, W], F32, tag="ot")
                    et1 = p_exp.tile([128, 4, W], F32, tag="et")
                    ot1 = ps_ot.tile([65, W], F32, tag="ot")
                    s0, a0, f0 = make_head(h0, et0, ot0)
                    s1, a1, f1 = make_head(h1, et1, ot1)
                    s0(0); s1(0)
                    s0(1); a0(0)
                    s1(1); a1(0)
                    s0(2); a0(1)
                    s1(2); a1(1)
                    s0(3); a0(2)
                    s1(3); a1(2)
                    a0(3); a1(3)
                    f0(); f1()

                # stage out_last^T columns: lastT[d, h] = outT[d, h, 511]
                nc.vector.tensor_copy(lastT[:, :], outt[:, :, W - 1])

                # out proj: out[tok, :] = sum_h outT[:, h, tok].T @ Wproj[h]
                pr = p_cp.tile([128, 4, C], F32)
                for t in range(4):
                    ps = ps_mm.tile([128, C], F32, tag="mm")
                    for h in range(H):
                        nc.tensor.matmul(
                            ps[:],
                            r(outt[:, h, t * 128:(t + 1) * 128]),
                            r(wproj[:, h, :]),
                            start=(h == 0), stop=(h == 7))
                    nc.vector.tensor_copy(pr[:, t, :], ps[:])
                nc.sync.dma_start(
                    out=out_ext[b].rearrange("(t p) c -> p t c", p=128),
                    in_=pr[:])

                # out_last natural row for this batch -> cc_in[b, h*64+d]
                cc_ap = cc_in[:]
                nc.sync.dma_start(
                    out=bass.AP(tensor=cc_ap.tensor, offset=b * C,
                                ap=[[1, 64], [64, H]]),
                    in_=lastT[:, :])

            # ================= collective =================
            nc.gpsimd.collective_compute(
                "AllGather",
                mybir.AluOpType.bypass,
                ins=[cc_in[:]],
                outs=[cc_out[:]],
                replica_groups=[list(range(N_CORES))],
            )

            # ================= decomposed phase =================
            ol = p_dec.tile([64, C], F32)  # out_last [64 batches, 512]
            nc.sync.dma_start(out=ol[:], in_=cc_out[:])
            lastT_all = persist.tile([128, 4, 64], BF16)
            for t in range(4):
                xp = ps_xp.tile([128, 64], F32, tag="xp")
                nc.tensor.transpose(
                    xp[:], ol[:, t * 128:(t + 1) * 128], ident[0:64, 0:64])
                nc.vector.tensor_copy(lastT_all[:, t, :], xp[:])

            for w in range(WPC):
                wc = p_wc.tile([128, 4, C], BF16)
                nc.scalar.dma_start(
                    out=wc[:],
                    in_=wctx_ext[:, w * C:(w + 1) * C].rearrange(
                        "(k p) c -> p k c", p=128))
                dps = ps_mm.tile([64, C], F32, tag="mm")
                for kc in range(4):
                    nc.tensor.matmul(
                        dps[0:64, :], lastT_all[:, kc, :], wc[:, kc, :],
                        start=(kc == 0), stop=(kc == 3))
                pv = p_dec.tile([64, C], F32)
                nc.sync.dma_start(out=pv[:], in_=prev_ext[:, w, :])
                s_sb = p_dec.tile([64, C], F32)
                nc.vector.tensor_add(s_sb[:], dps[0:64, :], pv[:])
                st = p_dec.tile([128, 4, 64], F32)
                xp = ps_xp.tile([128, 256], F32, tag="xp")
                for t in range(4):
                    nc.tensor.transpose(
                        xp[:, t * 64:(t + 1) * 64],
                        s_sb[:, t * 128:(t + 1) * 128],
                        ident[0:64, 0:64])
                nc.scalar.copy(r(st[:]), xp[:])
                d2 = ps_sc.tile([64, C], F32, tag="sc")
                for t in range(4):
                    nc.tensor.matmul(
                        d2[0:64, :], r(st[:, t, :]), r(wproj2[:, t, :]),
                        start=(t == 0), stop=(t == 3))
                d2s = p_dec.tile([64, C], F32)
                nc.scalar.copy(d2s[:], d2[0:64, :])
                nc.sync.dma_start(out=dec_ext[:, w, :], in_=d2s[:])

    nc.finalize()
    return nc


_NC_CACHE = None


def _get_nc():
    global _NC_CACHE
    if _NC_CACHE is None:
        _NC_CACHE = build_kernel()
    return _NC_CACHE


def make_in_maps(x, prev_decomposed, W_attn, W_ctx, W_proj, W_proj2):
    import ml_dtypes
    W_ctx = np.asarray(W_ctx).astype(ml_dtypes.bfloat16)
    ident = np.eye(128, dtype=np.float32)
    # scoresT layout [k, q]: keep k <= q within the diagonal block
    kk, qq = np.meshgrid(np.arange(128), np.arange(128), indexing="ij")
    maskt = np.where(kk > qq, np.float32(0.0), np.float32(1.0))
    ones = np.ones((128, 1), dtype=np.float32)

    in_maps = []
    for i in range(N_CORES):
        in_maps.append({
            "x": np.ascontiguousarray(x[i * BPC:(i + 1) * BPC]),
            "prev": np.ascontiguousarray(
                prev_decomposed[:, i * WPC:(i + 1) * WPC, :]),
            "w_attn": np.ascontiguousarray(W_attn),
            "w_ctx": np.ascontiguousarray(
                W_ctx[:, i * WPC * C:(i + 1) * WPC * C]),
            "w_proj": np.ascontiguousarray(W_proj),
            "w_proj2": np.ascontiguousarray(W_proj2),
            "ident": ident,
            "maskt": maskt,
            "ones": ones,
        })
    return in_maps


def run(x, prev_decomposed, W_attn, W_ctx, W_proj, W_proj2, **spmd_kwargs):
    nc = _get_nc()
    in_maps = make_in_maps(x, prev_decomposed, W_attn, W_ctx, W_proj, W_proj2)
    res = run_bass_kernel_spmd(nc, in_maps, list(range(N_CORES)), **spmd_kwargs)
    results = res.results
    out = np.concatenate(
        [np.asarray(results[i]["out"]) for i in range(N_CORES)], axis=0)
    dec = np.concatenate(
        [np.asarray(results[i]["dec"]) for i in range(N_CORES)], axis=1)
    return (out, dec), res


def kernel(x, prev_decomposed, W_attn, W_ctx, W_proj, W_proj2):
    (out, dec), _ = run(
        np.asarray(x, dtype=np.float32),
        np.asarray(prev_decomposed, dtype=np.float32),
        np.asarray(W_attn, dtype=np.float32),
        np.asarray(W_ctx, dtype=np.float32),
        np.asarray(W_proj, dtype=np.float32),
        np.asarray(W_proj2, dtype=np.float32))
    return (out, dec)
